# revision 1
# baseline (speedup 1.0000x reference)
"""Trainium2 Bass kernel for spatial self-attention (nn_Attention_90615220011343).

Module math (per batch b):
    qkv = x @ w_qkv            x:[N=4096, C=256], w_qkv:[256, 384]
    q,k,v -> heads (4 heads, dim 32)
    sim = (q*ds^-0.5) @ k^T    per head: [4096, 4096]
    attn = softmax(sim, -1)
    out = attn @ v             -> [N, 128]
    y = out @ w_out + b_out    -> [N, 256]

Sharding: 8 cores = 4 batches x 2 head-pairs. Core c -> batch c//2,
heads {2*(c%2), 2*(c%2)+1}. Each core computes a partial y (its two
heads' contribution); host sums the pair and adds b_out.

Per-core kernel layout strategy (all on-chip, no collectives):
  - x^T [2x128, 4096] via PE transposes (contraction dim C on partitions).
  - q^T replicated 4x along partitions (via host-tiled wq columns) so the
    K=32 sim matmuls can be packed 4-per-PE-pass with row tiling.
  - k^T stored "interleaved-stacked": j-chunk c (128 tokens) lives at
    partition base 32*(c%4), column block c//4. Any 3-4 consecutive
    chunks occupy distinct row-groups -> one row-tiled matmul group.
  - sim^T computed in [j, i] layout (j on partitions) so softmax exp is a
    pure elementwise pass (values are N(0,1); max-subtraction skipped -
    exp never overflows) and attn@v needs no transpose.
  - attn@v: lhsT = [v | 1] (ones column rides along, M=33) so row 32 of
    the psum accumulator is the softmax denominator for free.
  - normalization folded to the very end: y_h = (out_h @ w_out_h) scaled
    per-partition by 1/den_h, summed over the core's 2 heads on DVE.
"""

import numpy as np

HEADS = 4
DH = 32
N = 4096
C = 256
P = 128
NCH = 32  # number of 128-token j-chunks
ITILES = 8  # i tiles of 512
GROUPS = [4, 3, 4, 3, 4, 3, 4, 3, 4]  # j-chunks per sim/exp group (A/B slabs)

_CACHED = {}


def _build_nc():
    import concourse.bass as bass
    import concourse.mybir as mybir
    from concourse.tile import TileContext
    from concourse.masks import make_identity

    FP = mybir.dt.float32
    FR = mybir.dt.float32r
    AF = mybir.ActivationFunctionType
    ALU = mybir.AluOpType

    nc = bass.Bass(target_bir_lowering=False)
    x_d = nc.declare_dram_parameter("x", [N, C], FP, isOutput=False)
    wq_d = nc.declare_dram_parameter("wq", [C, 2 * P], FP, isOutput=False)
    wk_d = nc.declare_dram_parameter("wk", [C, 64], FP, isOutput=False)
    wv_d = nc.declare_dram_parameter("wv", [C, 64], FP, isOutput=False)
    wo_d = nc.declare_dram_parameter("wo", [64, C], FP, isOutput=False)
    y_d = nc.declare_dram_parameter("y", [N, C], FP, isOutput=True)

    with TileContext(nc) as tc:
        with (
            tc.tile_pool(name="const", bufs=1) as constp,
            tc.tile_pool(name="xin", bufs=10) as xinp,
            tc.tile_pool(name="big", bufs=1) as bigp,
            tc.tile_pool(name="exp", bufs=2) as expp,
            tc.tile_pool(name="ytmp", bufs=4) as ytmpp,
            tc.tile_pool(name="psA", bufs=1, space="PSUM") as psA,
            tc.tile_pool(name="psB", bufs=1, space="PSUM") as psB,
            tc.tile_pool(name="psV", bufs=1, space="PSUM") as psV,
        ):
            ident = constp.tile([P, P], FP, tag="ident")
            make_identity(nc, ident[:])

            # ---- persistent SBUF tensors ----
            xT = [bigp.tile([P, N], FR, tag=f"xT{cc}", name=f"xT{cc}") for cc in range(2)]
            qrep = [bigp.tile([P, N], FR, tag=f"qrep{h}", name=f"qrep{h}") for h in range(2)]
            karr = [bigp.tile([P, N // 4], FR, tag=f"karr{h}", name=f"karr{h}") for h in range(2)]
            vaug = [bigp.tile([P, 33 * NCH], FR, tag=f"vaug{h}", name=f"vaug{h}") for h in range(2)]
            outT = bigp.tile([64, N], FR, tag="outT")
            # softmax denominators: head h at partition 32*h
            denrow = bigp.tile([33, N], FP, tag="denrow")
            rden = bigp.tile([P, 64], FP, tag="rden")
            wq_sb = bigp.tile([P, 2, 2 * P], FR, tag="wq")
            wk_sb = bigp.tile([P, 2, 64], FR, tag="wk")
            wv_sb = bigp.tile([P, 2, 64], FR, tag="wv")
            wo_sb = bigp.tile([64, C], FR, tag="wo")

            # ---- weight + x loads (stage fp32, round to fp32r on DVE) ----
            wq_st = bigp.tile([P, 2, 2 * P], FP, tag="wq_st")
            wk_st = bigp.tile([P, 2, 64], FP, tag="wk_st")
            wv_st = bigp.tile([P, 2, 64], FP, tag="wv_st")
            wo_st = bigp.tile([64, C], FP, tag="wo_st")
            for cc in range(2):
                nc.sync.dma_start(out=wq_st[:, cc, :], in_=wq_d[cc * P:(cc + 1) * P, :])
                nc.sync.dma_start(out=wk_st[:, cc, :], in_=wk_d[cc * P:(cc + 1) * P, :])
                nc.sync.dma_start(out=wv_st[:, cc, :], in_=wv_d[cc * P:(cc + 1) * P, :])
            nc.sync.dma_start(out=wo_st[:], in_=wo_d[:])
            nc.vector.tensor_copy(out=wq_sb[:], in_=wq_st[:])
            nc.vector.tensor_copy(out=wk_sb[:], in_=wk_st[:])
            nc.vector.tensor_copy(out=wv_sb[:], in_=wv_st[:])
            nc.vector.tensor_copy(out=wo_sb[:], in_=wo_st[:])

            # ---- x load + transpose to xT ----
            # rounds: (pool, tag, nk list); each slab holds both c-chunks of
            # its nk's interleaved: [nk0/cc0, nk0/cc1, nk1/cc0, ...]
            tp_rounds = [
                (psA, "A", list(range(0, 8))),
                (psB, "B", list(range(8, 14))),
                (psA, "A", list(range(14, 22))),
                (psB, "B", list(range(22, 28))),
                (psA, "A", list(range(28, 32))),
            ]
            for pool, tag, nks in tp_rounds:
                L = 2048 if tag == "A" else 1536
                slab = pool.tile([P, L], FP, tag=tag)
                for i, nk in enumerate(nks):
                    xt = xinp.tile([P, C], FP, tag="xt")
                    dmae = nc.sync if nk % 2 == 0 else nc.scalar
                    dmae.dma_start(out=xt[:], in_=x_d[P * nk:P * (nk + 1), :])
                    for cc in range(2):
                        nc.tensor.transpose(
                            slab[:, 256 * i + P * cc: 256 * i + P * (cc + 1)],
                            xt[:, P * cc:P * (cc + 1)],
                            ident[:],
                        )
                n = len(nks)
                sv = slab[:].rearrange("p (k c f) -> p k c f", c=2, f=P)
                for cc in range(2):
                    nc.vector.tensor_copy(
                        out=xT[cc][:, P * nks[0]: P * (nks[0] + n)],
                        in_=sv[:, 0:n, cc, :],
                    )

            # ---- qkv projections ----
            def qrep_rounds(h):
                for pool, tag, it0, nits in (
                    (psA, "A", 0, 4), (psB, "B", 4, 3), (psA, "A", 7, 1),
                ):
                    L = 2048 if tag == "A" else 1536
                    slab = pool.tile([P, L], FP, tag=tag)
                    for cc in range(2):
                        for r in range(nits):
                            it = it0 + r
                            nc.tensor.matmul(
                                slab[:, 512 * r: 512 * (r + 1)],
                                lhsT=wq_sb[:, cc, P * h: P * (h + 1)],
                                rhs=xT[cc][:, 512 * it: 512 * (it + 1)],
                                start=(cc == 0), stop=(cc == 1),
                            )
                    nc.vector.tensor_copy(
                        out=qrep[h][:, 512 * it0: 512 * (it0 + nits)],
                        in_=slab[:, : 512 * nits],
                    )

            def karr_build(h):
                # karr[32*(c%4) : +32, 128*(c//4) : +128] = k^T of j-chunk c
                # (col-tiling is incompatible with fp32r: all matmuls write
                # partition base 0; DVE relocates to the stacked layout)
                for p_ in range(2):
                    slab = psA.tile([P, 2048], FP, tag="A")
                    for ct in range(4):
                        # rhs: j-chunks c = 4m+ct for m in [4p, 4p+4) -> strided view
                        for cc in range(2):
                            xv = xT[cc][:].rearrange(
                                "q (m t f) -> q m t f", t=4, f=P
                            )[:, 4 * p_: 4 * p_ + 4, ct, :]
                            nc.tensor.matmul(
                                slab[0:32, 512 * ct: 512 * (ct + 1)],
                                lhsT=wk_sb[:, cc, 32 * h: 32 * (h + 1)],
                                rhs=xv,
                                start=(cc == 0), stop=(cc == 1),
                            )
                    for ct in range(4):
                        nc.vector.tensor_copy(
                            out=karr[h][32 * ct: 32 * (ct + 1), 512 * p_: 512 * (p_ + 1)],
                            in_=slab[0:32, 512 * ct: 512 * (ct + 1)],
                        )

            def v_build():
                # both heads at once: psum [128, 64*(k%..)] chunks
                slab = psA.tile([P, 2048], FP, tag="A")
                for k in range(NCH):
                    for cc in range(2):
                        nc.tensor.matmul(
                            slab[:, 64 * k: 64 * (k + 1)],
                            lhsT=xT[cc][:, P * k: P * (k + 1)],
                            rhs=wv_sb[:, cc, :],
                            start=(cc == 0), stop=(cc == 1),
                        )
                sv = slab[:].rearrange("p (k d) -> p k d", d=64)
                ones_st = bigp.tile([P, NCH], FP, tag="ones_st")
                nc.gpsimd.memset(ones_st[:], 1.0)
                for h in range(2):
                    vv = vaug[h][:].rearrange("p (k e) -> p k e", e=33)
                    nc.vector.tensor_copy(out=vv[:, :, 32], in_=ones_st[:])
                    nc.vector.tensor_copy(
                        out=vv[:, :, 0:32], in_=sv[:, :, 32 * h: 32 * (h + 1)]
                    )

            qrep_rounds(0)
            karr_build(0)
            v_build()

            # head-0 projection accumulator (filled during head-1 attention)
            yacc = bigp.tile([P, NCH * C], FP, tag="yacc")
            yv = yacc[:].rearrange("p (k c) -> p k c", c=C)

            def attention(h, post_it=None):
                vv = vaug[h][:].rearrange("p (k e) -> p k e", e=33)
                for it in range(ITILES):
                    i0 = 512 * it
                    av = psV.tile([P, 512], FP, tag="V")
                    cstart = 0
                    for gsz in GROUPS:
                        pool, tag, L = (psA, "A", 2048) if gsz == 4 else (psB, "B", 1536)
                        slab = pool.tile([P, L], FP, tag=tag)
                        for r in range(gsz):
                            c = cstart + r
                            rt = c % 4
                            nc.tensor.matmul(
                                slab[:, 512 * r: 512 * (r + 1)],
                                lhsT=karr[h][32 * rt: 32 * (rt + 1), P * (c // 4): P * (c // 4 + 1)],
                                rhs=qrep[h][32 * rt: 32 * (rt + 1), i0: i0 + 512],
                                start=True, stop=True,
                                tile_position=(32 * rt, 0),
                            )
                        eslab = expp.tile([P, L], FR, tag="E")
                        nc.scalar.activation(eslab[:], slab[:], AF.Exp)
                        for r in range(gsz):
                            c = cstart + r
                            nc.tensor.matmul(
                                av[0:33, :],
                                lhsT=vv[:, c, :],
                                rhs=eslab[:, 512 * r: 512 * (r + 1)],
                                start=(c == 0), stop=(c == NCH - 1),
                                skip_group_check=True,
                            )
                        cstart += gsz
                    nc.vector.tensor_copy(out=outT[32 * h: 32 * h + 32, i0: i0 + 512], in_=av[0:32, :])
                    nc.vector.tensor_copy(out=denrow[32 * h: 32 * h + 1, i0: i0 + 512], in_=av[32:33, :])
                    if post_it is not None:
                        post_it(it)

            def den_recip(h):
                # denominator row -> column layout, reciprocal
                dslab = psV.tile([P, 512], FP, tag="V")
                for t in range(NCH):
                    nc.tensor.transpose(
                        dslab[:, t: t + 1],
                        denrow[32 * h: 32 * h + 1, P * t: P * (t + 1)],
                        ident[32 * h: 32 * h + 1, 32 * h: 32 * h + 1],
                    )
                nc.vector.reciprocal(out=rden[:, 32 * h: 32 * h + 32], in_=dslab[:, 0:32])

            def y0_chunks(it):
                # head-0 output projection, interleaved into head-1 attention
                for k in range(4 * it, 4 * it + 4):
                    yp = psV.tile([P, 512], FP, tag="V")
                    nc.tensor.matmul(
                        yp[:, 0:C], lhsT=outT[0:32, P * k: P * (k + 1)],
                        rhs=wo_sb[0:32, :],
                        start=True, stop=True, tile_position=(0, 0),
                    )
                    nc.vector.tensor_scalar_mul(yv[:, k, :], yp[:, 0:C], rden[:, k: k + 1])

            attention(0)
            den_recip(0)
            qrep_rounds(1)
            karr_build(1)
            attention(1, post_it=y0_chunks)
            den_recip(1)

            # ---- tail: head-1 projection + combine + store ----
            for k in range(NCH):
                pool, tag, L = (psA, "A", 2048) if k % 2 == 0 else (psB, "B", 1536)
                yb = pool.tile([P, L], FP, tag=tag)
                nc.tensor.matmul(
                    yb[:, 0:C], lhsT=outT[32:64, P * k: P * (k + 1)],
                    rhs=wo_sb[32:64, :],
                    start=True, stop=True, tile_position=(32, 0),
                )
                yo = ytmpp.tile([P, C], FP, tag="yo")
                nc.vector.scalar_tensor_tensor(
                    out=yo[:], in0=yb[:, 0:C], scalar=rden[:, 32 + k: 33 + k],
                    in1=yv[:, k, :], op0=ALU.mult, op1=ALU.add,
                )
                dmae = nc.sync if k % 2 == 0 else nc.scalar
                dmae.dma_start(out=y_d[P * k: P * (k + 1), :], in_=yo[:])

    _split_excess_waits(nc, mybir)
    return nc


def _split_excess_waits(nc, mybir, maxw=1, carrier_cap=1):
    """walrus codegen allows few semaphore waits per engine instruction.

    Tile's scheduler can emit 3-4 on one matmul. Hoist the excess onto
    InstEventSemaphore carriers inserted immediately before the instruction
    on the same engine queue (queue is FIFO, so waiting in the carrier is
    equivalent; no reordering so no deadlock risk).
    """
    skip = {
        "InstEventSemaphore", "InstCall",
        "InstUnconditionalBranch", "InstISA", "InstRegisterMove",
    }
    for f in nc.m.functions:
        for blk in f.blocks:
            idx = 0
            while idx < len(blk.instructions):
                ins = blk.instructions[idx]
                si = getattr(ins, "sync_info", None)
                if (
                    si is not None and si.on_wait and len(si.on_wait) > maxw
                    and type(ins).__name__ not in skip
                ):
                    waits = list(si.on_wait)
                    keep, excess = waits[:maxw], waits[maxw:]
                    n_ins = 0
                    for i in range(0, len(excess), carrier_cap):
                        ev = mybir.InstEventSemaphore(
                            name=nc.get_next_instruction_name(),
                            engine=ins.engine,
                            ins=[], outs=[],
                            sync_info=mybir.SyncInfo(
                                on_wait=excess[i:i + carrier_cap], on_update=[]
                            ),
                        )
                        nc.register_instruction(ev)
                        blk.instructions.insert(idx + n_ins, ev)
                        n_ins += 1
                    ins.sync_info = mybir.SyncInfo(
                        on_wait=keep, on_update=list(si.on_update or [])
                    )
                    idx += n_ins
                idx += 1
    return nc


def get_nc():
    if "nc" not in _CACHED:
        _CACHED["nc"] = _build_nc()
    return _CACHED["nc"]


def make_in_maps(x, w_qkv, w_out):
    """Host-side sharding: core c -> batch c//2, heads (c%2)*2, (c%2)*2+1."""
    B = x.shape[0]
    xf = np.ascontiguousarray(x.reshape(B, N, C))
    scale = DH ** -0.5
    in_maps = []
    for core in range(8):
        b, hp = core // 2, core % 2
        h0, h1 = 2 * hp, 2 * hp + 1
        wq = np.concatenate(
            [np.tile(w_qkv[:, h * DH:(h + 1) * DH] * scale, (1, 4)) for h in (h0, h1)],
            axis=1,
        )  # [256, 256]
        wk = np.concatenate(
            [w_qkv[:, 128 + h * DH: 128 + (h + 1) * DH] for h in (h0, h1)], axis=1
        )  # [256, 64]
        wv = np.concatenate(
            [w_qkv[:, 256 + h * DH: 256 + (h + 1) * DH] for h in (h0, h1)], axis=1
        )  # [256, 64]
        wo = np.concatenate(
            [w_out[h * DH:(h + 1) * DH, :] for h in (h0, h1)], axis=0
        )  # [64, 256]
        in_maps.append({
            "x": np.ascontiguousarray(xf[b]),
            "wq": np.ascontiguousarray(wq.astype(np.float32)),
            "wk": np.ascontiguousarray(wk.astype(np.float32)),
            "wv": np.ascontiguousarray(wv.astype(np.float32)),
            "wo": np.ascontiguousarray(wo.astype(np.float32)),
        })
    return in_maps


def kernel(x, w_qkv, w_out, b_out):
    from concourse.bass_utils import run_bass_kernel_spmd

    nc = get_nc()
    in_maps = make_in_maps(
        np.asarray(x, dtype=np.float32),
        np.asarray(w_qkv, dtype=np.float32),
        np.asarray(w_out, dtype=np.float32),
    )
    res = run_bass_kernel_spmd(nc, in_maps, list(range(8))).results
    B, H, W = 4, 64, 64
    y = np.empty((B, N, C), dtype=np.float32)
    for b in range(B):
        y[b] = res[2 * b]["y"] + res[2 * b + 1]["y"]
    y += np.asarray(b_out, dtype=np.float32)
    return y.reshape(B, H, W, C)



# revision 11
# speedup vs baseline: 1.2017x; 1.2017x over previous
"""Trainium2 Bass kernel for spatial self-attention (nn_Attention_90615220011343).

Module math (per batch b):
    qkv = x @ w_qkv            x:[N=4096, C=256], w_qkv:[256, 384]
    q,k,v -> heads (4 heads, dim 32)
    sim = (q*ds^-0.5) @ k^T    per head: [4096, 4096]
    attn = softmax(sim, -1)
    out = attn @ v             -> [N, 128]
    y = out @ w_out + b_out    -> [N, 256]

Sharding: 8 cores = 4 batches x 2 head-pairs. Core c -> batch c//2,
heads {2*(c%2), 2*(c%2)+1}. Each core computes a partial y (its two
heads' contribution); host sums the pair and adds b_out.

Per-core layout (all on-chip, no collectives). The Activation engine is
the roofline (33.5M softmax exps / 128 lanes); everything else is
organized to keep it saturated:
  - x^T [2x128, 4096] via PE transposes (contraction dim C on partitions).
  - q^T, k^T stored flat [32, 4096] fp32r (contract dim 32 on partitions
    0-31; sim matmuls stream 512-col i-tiles at 1 cycle/row).
  - sim^T computed in [j, i] psum slabs (A: 4 banks / B: 3 banks,
    ping-pong) so exp is one big Activation op per slab, no reductions.
  - exp output in fp16; attn@v flipped to out[i, d]: lhsT = exp-slab
    [j, 128-i-block], rhs = [v_h | 1] fp16 [j, 33] -> 33-row matmuls
    accumulating [128, 33] per i-block in psum, denominator rides in
    column 32.  4x fewer PE rows than the [d, i] orientation.
  - per i-tile: reciprocal + scale (DVE), PE-transpose of the normalized
    [128i, 4x32d] block into outT [64, 4096] fp16.
  - y = outT-block^T @ w_out contracts both heads at once (fp16), riding
    the A-slab psum ring lagged one i-tile behind attention.
"""

import numpy as np

HEADS = 4
DH = 32
N = 4096
C = 256
P = 128
NCH = 32  # number of 128-token j-chunks
ITILES = 8  # i tiles of 512
GROUPS = [4, 3, 4, 3, 4, 3, 4, 3, 4]  # j-chunks per sim/exp group (A/B slabs)

_CACHED = {}


def _build_nc():
    import concourse.bass as bass
    import concourse.mybir as mybir
    from concourse.tile import TileContext
    from concourse.masks import make_identity

    FP = mybir.dt.float32
    FR = mybir.dt.float32r
    F16 = mybir.dt.float16
    AF = mybir.ActivationFunctionType

    nc = bass.Bass(target_bir_lowering=False)
    x_d = nc.declare_dram_parameter("x", [N, C], FP, isOutput=False)
    wq_d = nc.declare_dram_parameter("wq", [C, 64], FP, isOutput=False)
    wk_d = nc.declare_dram_parameter("wk", [C, 64], FP, isOutput=False)
    wv_d = nc.declare_dram_parameter("wv", [C, 64], FP, isOutput=False)
    wo_d = nc.declare_dram_parameter("wo", [64, C], FP, isOutput=False)
    y_d = nc.declare_dram_parameter("y", [N, C], FP, isOutput=True)

    with TileContext(nc) as tc:
        with (
            tc.tile_pool(name="const", bufs=1) as constp,
            tc.tile_pool(name="xin", bufs=10) as xinp,
            tc.tile_pool(name="big", bufs=1) as bigp,
            tc.tile_pool(name="exp", bufs=3) as expp,
            tc.tile_pool(name="stg", bufs=2) as stgp,
            tc.tile_pool(name="ytmp", bufs=4) as ytmpp,
            tc.tile_pool(name="psA", bufs=1, space="PSUM") as psA,
            tc.tile_pool(name="psB", bufs=1, space="PSUM") as psB,
            tc.tile_pool(name="psV", bufs=1, space="PSUM") as psV,
        ):
            ident = constp.tile([P, P], FP, tag="ident")
            make_identity(nc, ident[:])

            # ---- persistent SBUF tensors ----
            xT = [bigp.tile([P, N], FR, tag=f"xT{cc}", name=f"xT{cc}") for cc in range(2)]
            qT = [bigp.tile([32, N], FR, tag=f"qT{h}", name=f"qT{h}") for h in range(2)]
            karr = [bigp.tile([32, N], FR, tag=f"karr{h}", name=f"karr{h}") for h in range(2)]
            vaug = [bigp.tile([P, 33 * NCH], F16, tag=f"vaug{h}", name=f"vaug{h}") for h in range(2)]
            outT = bigp.tile([64, N], F16, tag="outT")
            rden = bigp.tile([P, 8 * ITILES], FP, tag="rden")
            wq_sb = bigp.tile([P, 2, 64], FR, tag="wq")
            wk_sb = bigp.tile([P, 2, 64], FR, tag="wk")
            wv_sb = bigp.tile([P, 2, 64], FR, tag="wv")
            wo_sb = bigp.tile([64, C], F16, tag="wo")

            # ---- weight loads (gpsimd queue; DVE rounds to fp32r / fp16) ----
            wq_st = bigp.tile([P, 2, 64], FP, tag="wq_st")
            wk_st = bigp.tile([P, 2, 64], FP, tag="wk_st")
            wv_st = bigp.tile([P, 2, 64], FP, tag="wv_st")
            wo_st = bigp.tile([64, C], FP, tag="wo_st")
            for cc in range(2):
                nc.gpsimd.dma_start(out=wq_st[:, cc, :], in_=wq_d[cc * P:(cc + 1) * P, :])
                nc.gpsimd.dma_start(out=wk_st[:, cc, :], in_=wk_d[cc * P:(cc + 1) * P, :])
                nc.gpsimd.dma_start(out=wv_st[:, cc, :], in_=wv_d[cc * P:(cc + 1) * P, :])
            nc.gpsimd.dma_start(out=wo_st[:], in_=wo_d[:])
            nc.vector.tensor_copy(out=wq_sb[:], in_=wq_st[:])
            nc.vector.tensor_copy(out=wk_sb[:], in_=wk_st[:])
            nc.vector.tensor_copy(out=wv_sb[:], in_=wv_st[:])
            nc.vector.tensor_copy(out=wo_sb[:], in_=wo_st[:])

            # ---- x load + transpose to xT ----
            # rounds: (pool, tag, nk list); each slab holds both c-chunks of
            # its nk's interleaved: [nk0/cc0, nk0/cc1, nk1/cc0, ...]
            tp_rounds = [
                (psA, "A", list(range(0, 8))),
                (psB, "B", list(range(8, 14))),
                (psA, "A", list(range(14, 22))),
                (psB, "B", list(range(22, 28))),
                (psA, "A", list(range(28, 32))),
            ]
            for pool, tag, nks in tp_rounds:
                L = 2048 if tag == "A" else 1536
                slab = pool.tile([P, L], FP, tag=tag)
                for i, nk in enumerate(nks):
                    xt = xinp.tile([P, C], FP, tag="xt")
                    dmae = nc.sync if nk % 2 == 0 else nc.gpsimd
                    dmae.dma_start(out=xt[:], in_=x_d[P * nk:P * (nk + 1), :])
                    for cc in range(2):
                        nc.tensor.transpose(
                            slab[:, 256 * i + P * cc: 256 * i + P * (cc + 1)],
                            xt[:, P * cc:P * (cc + 1)],
                            ident[:],
                        )
                n = len(nks)
                sv = slab[:].rearrange("p (k c f) -> p k c f", c=2, f=P)
                for cc in range(2):
                    nc.vector.tensor_copy(
                        out=xT[cc][:, P * nks[0]: P * (nks[0] + n)],
                        in_=sv[:, 0:n, cc, :],
                    )

            # ---- qkv projections ----
            def proj_round(w_sb, h, dst, pool, tag, it0, nits):
                # dst[32, cols] = (x @ w[:, 32h:32h+32])^T, via psum [0:32, :]
                L = 2048 if tag == "A" else 1536
                slab = pool.tile([P, L], FP, tag=tag)
                for r in range(nits):
                    it = it0 + r
                    for cc in range(2):
                        nc.tensor.matmul(
                            slab[0:32, 512 * r: 512 * (r + 1)],
                            lhsT=w_sb[:, cc, 32 * h: 32 * (h + 1)],
                            rhs=xT[cc][:, 512 * it: 512 * (it + 1)],
                            start=(cc == 0), stop=(cc == 1),
                        )
                nc.vector.tensor_copy(
                    out=dst[:, 512 * it0: 512 * (it0 + nits)],
                    in_=slab[0:32, : 512 * nits],
                )

            def proj_rounds(w_sb, h, dst):
                proj_round(w_sb, h, dst, psA, "A", 0, 4)
                proj_round(w_sb, h, dst, psB, "B", 4, 3)
                proj_round(w_sb, h, dst, psA, "A", 7, 1)

            def v_build():
                # both heads at once: psum [128, 64] chunks packed in A slab
                slab = psA.tile([P, 2048], FP, tag="A")
                for k in range(NCH):
                    for cc in range(2):
                        nc.tensor.matmul(
                            slab[:, 64 * k: 64 * (k + 1)],
                            lhsT=xT[cc][:, P * k: P * (k + 1)],
                            rhs=wv_sb[:, cc, :],
                            start=(cc == 0), stop=(cc == 1),
                        )
                sv = slab[:].rearrange("p (k d) -> p k d", d=64)
                for h in range(2):
                    vv = vaug[h][:].rearrange("p (k e) -> p k e", e=33)
                    nc.vector.memset(vv[:, :, 32], 1.0)
                    nc.vector.tensor_copy(
                        out=vv[:, :, 0:32], in_=sv[:, :, 32 * h: 32 * (h + 1)]
                    )

            # head-0 inputs built up front; head-1 projections are injected
            # into attention(0)'s early i-tiles (PE slack under the exp
            # roofline) so the first exp isn't delayed.
            proj_rounds(wk_sb, 0, karr[0][:])
            proj_rounds(wq_sb, 0, qT[0][:])
            v_build()
            builds = [
                lambda: proj_round(wk_sb, 1, karr[1][:], psA, "A", 0, 4),
                lambda: (
                    proj_round(wk_sb, 1, karr[1][:], psB, "B", 4, 3),
                    proj_round(wk_sb, 1, karr[1][:], psA, "A", 7, 1),
                ),
                lambda: proj_round(wq_sb, 1, qT[1][:], psA, "A", 0, 4),
                lambda: (
                    proj_round(wq_sb, 1, qT[1][:], psB, "B", 4, 3),
                    proj_round(wq_sb, 1, qT[1][:], psA, "A", 7, 1),
                ),
            ]

            # ---- attention ----
            # per (h, it): sim slabs -> exp (fp16) -> attn@v accumulating
            # av[128i, 33]x4 blocks in the V bank (den in col 32); then
            # recip+scale (DVE), PE-transpose into outT[32h:+32, i-tile].
            def y_proj(it):
                # y rides the A-slab ring: 4 blocks x [128, 256] in one gen
                i0 = 512 * it
                yslab = psA.tile([P, 2048], FP, tag="A")
                for m in range(4):
                    nc.tensor.matmul(
                        yslab[:, 256 * m: 256 * (m + 1)],
                        lhsT=outT[0:64, i0 + P * m: i0 + P * (m + 1)],
                        rhs=wo_sb[:],
                        start=True, stop=True, skip_group_check=True,
                    )
                for m in range(4):
                    yo = ytmpp.tile([P, C], FP, tag="yo")
                    nc.vector.tensor_copy(out=yo[:], in_=yslab[:, 256 * m: 256 * (m + 1)])
                    nc.sync.dma_start(
                        out=y_d[i0 + P * m: i0 + P * (m + 1), :], in_=yo[:]
                    )

            def attention(h, with_y, pre=None):
                vv = vaug[h][:].rearrange("p (k e) -> p k e", e=33)
                for it in range(ITILES):
                    i0 = 512 * it
                    if pre and it > 0:
                        pre.pop(0)()
                    if with_y and it > 0:
                        y_proj(it - 1)
                    # V bank tile: cols 0-131 av (4 blocks x 33), 132-259
                    # transpose scratch; disjoint byte ranges within one gen
                    vt = psV.tile([P, 260], FP, tag="V")
                    avt = vt[:, 0:132]
                    av = avt.rearrange("p (m e) -> p m e", e=33)
                    cstart = 0
                    for gsz in GROUPS:
                        pool, tag, L = (psA, "A", 2048) if gsz == 4 else (psB, "B", 1536)
                        slab = pool.tile([P, L], FP, tag=tag)
                        for r in range(gsz):
                            c = cstart + r
                            nc.tensor.matmul(
                                slab[:, 512 * r: 512 * (r + 1)],
                                lhsT=karr[h][:, P * c: P * (c + 1)],
                                rhs=qT[h][:, i0: i0 + 512],
                                start=True, stop=True,
                            )
                        eslab = expp.tile([P, L], F16, tag="E")
                        nc.scalar.activation(eslab[:], slab[:], AF.Exp)
                        for r in range(gsz):
                            c = cstart + r
                            for m in range(4):
                                nc.tensor.matmul(
                                    avt[:, 33 * m: 33 * (m + 1)],
                                    lhsT=eslab[:, 512 * r + P * m: 512 * r + P * (m + 1)],
                                    rhs=vv[:, c, :],
                                    start=(c == 0 and m == 0),
                                    stop=(c == NCH - 1 and m == 3),
                                    skip_group_check=True,
                                )
                        cstart += gsz
                    # post: reciprocal of dens, normalize, transpose to outT
                    rd = rden[:, 8 * it + 4 * h: 8 * it + 4 * h + 4]
                    nc.vector.reciprocal(out=rd, in_=av[:, :, 32])
                    stg = stgp.tile([P, P], FP, tag="s")
                    for m in range(4):
                        nc.vector.tensor_scalar_mul(
                            stg[:, 32 * m: 32 * (m + 1)], av[:, m, 0:32],
                            rd[:, m: m + 1],
                        )
                    nc.tensor.matmul(
                        vt[:, 132:260], lhsT=stg[:], rhs=ident[:],
                        is_transpose=True, start=True, stop=True,
                        skip_group_check=True,
                    )
                    for m in range(4):
                        nc.vector.tensor_copy(
                            out=outT[32 * h: 32 * h + 32, i0 + P * m: i0 + P * (m + 1)],
                            in_=vt[32 * m: 32 * (m + 1), 132:260],
                        )

            attention(0, with_y=False, pre=builds)
            attention(1, with_y=True)
            y_proj(ITILES - 1)

    _split_excess_waits(nc, mybir)
    return nc


def _split_excess_waits(nc, mybir, maxw=1, carrier_cap=1):
    """walrus codegen allows few semaphore waits per engine instruction.

    Tile's scheduler can emit 3-4 on one matmul. Hoist the excess onto
    InstEventSemaphore carriers inserted immediately before the instruction
    on the same engine queue (queue is FIFO, so waiting in the carrier is
    equivalent; no reordering so no deadlock risk).
    """
    skip = {
        "InstEventSemaphore", "InstCall",
        "InstUnconditionalBranch", "InstISA", "InstRegisterMove",
    }
    for f in nc.m.functions:
        for blk in f.blocks:
            idx = 0
            while idx < len(blk.instructions):
                ins = blk.instructions[idx]
                si = getattr(ins, "sync_info", None)
                if (
                    si is not None and si.on_wait and len(si.on_wait) > maxw
                    and type(ins).__name__ not in skip
                ):
                    waits = list(si.on_wait)
                    keep, excess = waits[:maxw], waits[maxw:]
                    n_ins = 0
                    for i in range(0, len(excess), carrier_cap):
                        ev = mybir.InstEventSemaphore(
                            name=nc.get_next_instruction_name(),
                            engine=ins.engine,
                            ins=[], outs=[],
                            sync_info=mybir.SyncInfo(
                                on_wait=excess[i:i + carrier_cap], on_update=[]
                            ),
                        )
                        nc.register_instruction(ev)
                        blk.instructions.insert(idx + n_ins, ev)
                        n_ins += 1
                    ins.sync_info = mybir.SyncInfo(
                        on_wait=keep, on_update=list(si.on_update or [])
                    )
                    idx += n_ins
                idx += 1
    return nc


def get_nc():
    if "nc" not in _CACHED:
        _CACHED["nc"] = _build_nc()
    return _CACHED["nc"]


def make_in_maps(x, w_qkv, w_out):
    """Host-side sharding: core c -> batch c//2, heads (c%2)*2, (c%2)*2+1."""
    B = x.shape[0]
    xf = np.ascontiguousarray(x.reshape(B, N, C))
    scale = DH ** -0.5
    in_maps = []
    for core in range(8):
        b, hp = core // 2, core % 2
        h0, h1 = 2 * hp, 2 * hp + 1
        wq = np.concatenate(
            [w_qkv[:, h * DH:(h + 1) * DH] * scale for h in (h0, h1)], axis=1
        )  # [256, 64]
        wk = np.concatenate(
            [w_qkv[:, 128 + h * DH: 128 + (h + 1) * DH] for h in (h0, h1)], axis=1
        )  # [256, 64]
        wv = np.concatenate(
            [w_qkv[:, 256 + h * DH: 256 + (h + 1) * DH] for h in (h0, h1)], axis=1
        )  # [256, 64]
        wo = np.concatenate(
            [w_out[h * DH:(h + 1) * DH, :] for h in (h0, h1)], axis=0
        )  # [64, 256]
        in_maps.append({
            "x": np.ascontiguousarray(xf[b]),
            "wq": np.ascontiguousarray(wq.astype(np.float32)),
            "wk": np.ascontiguousarray(wk.astype(np.float32)),
            "wv": np.ascontiguousarray(wv.astype(np.float32)),
            "wo": np.ascontiguousarray(wo.astype(np.float32)),
        })
    return in_maps


def kernel(x, w_qkv, w_out, b_out):
    from concourse.bass_utils import run_bass_kernel_spmd

    nc = get_nc()
    in_maps = make_in_maps(
        np.asarray(x, dtype=np.float32),
        np.asarray(w_qkv, dtype=np.float32),
        np.asarray(w_out, dtype=np.float32),
    )
    res = run_bass_kernel_spmd(nc, in_maps, list(range(8))).results
    B, H, W = 4, 64, 64
    y = np.empty((B, N, C), dtype=np.float32)
    for b in range(B):
        y[b] = res[2 * b]["y"] + res[2 * b + 1]["y"]
    y += np.asarray(b_out, dtype=np.float32)
    return y.reshape(B, H, W, C)


# revision 16
# speedup vs baseline: 1.3814x; 1.1495x over previous
"""Trainium2 Bass kernel for spatial self-attention (nn_Attention_90615220011343).

Module math (per batch b):
    qkv = x @ w_qkv            x:[N=4096, C=256], w_qkv:[256, 384]
    q,k,v -> heads (4 heads, dim 32)
    sim = (q*ds^-0.5) @ k^T    per head: [4096, 4096]
    attn = softmax(sim, -1)
    out = attn @ v             -> [N, 128]
    y = out @ w_out + b_out    -> [N, 256]

Sharding: 8 cores = 4 batches x 2 head-pairs. Core c -> batch c//2,
heads {2*(c%2), 2*(c%2)+1}. Each core computes a partial y (its two
heads' contribution); host sums the pair and adds b_out.

Per-core layout (all on-chip, no collectives). The Activation engine is
the roofline (33.5M softmax exps / 128 lanes); everything else is
organized to keep it saturated:
  - x^T [2x128, 4096] via PE transposes (contraction dim C on partitions).
  - q^T, k^T stored flat [32, 4096] fp32r (contract dim 32 on partitions
    0-31; sim matmuls stream 512-col i-tiles at 1 cycle/row).
  - sim^T computed in [j, i] psum slabs (A: 4 banks / B: 3 banks,
    ping-pong) so exp is one big Activation op per slab, no reductions.
  - exp output in fp16; attn@v flipped to out[i, d]: lhsT = exp-slab
    [j, 128-i-block], rhs = [v_h | 1] fp16 [j, 33] -> 33-row matmuls
    accumulating [128, 33] per i-block in psum, denominator rides in
    column 32.  4x fewer PE rows than the [d, i] orientation.
  - per i-tile: reciprocal + scale (DVE), PE-transpose of the normalized
    [128i, 4x32d] block into outT [64, 4096] fp16.
  - y = outT-block^T @ w_out contracts both heads at once (fp16), riding
    the A-slab psum ring lagged one i-tile behind attention.
"""

import numpy as np

HEADS = 4
DH = 32
N = 4096
C = 256
P = 128
NCH = 32  # number of 128-token j-chunks
ITILES = 8  # i tiles of 512
GROUPS = [4, 3, 4, 3, 4, 3, 4, 3, 4]  # j-chunks per sim/exp group (A/B slabs)

_CACHED = {}


def _build_nc():
    import concourse.bass as bass
    import concourse.mybir as mybir
    from concourse.tile import TileContext
    from concourse.masks import make_identity

    FP = mybir.dt.float32
    FR = mybir.dt.float32r
    F16 = mybir.dt.float16
    AF = mybir.ActivationFunctionType

    nc = bass.Bass(target_bir_lowering=False)
    x_d = nc.declare_dram_parameter("x", [N, C], FP, isOutput=False)
    wq_d = nc.declare_dram_parameter("wq", [C, 64], FP, isOutput=False)
    wk_d = nc.declare_dram_parameter("wk", [C, 64], FP, isOutput=False)
    wv_d = nc.declare_dram_parameter("wv", [C, 64], FP, isOutput=False)
    wo_d = nc.declare_dram_parameter("wo", [64, C], FP, isOutput=False)
    y_d = nc.declare_dram_parameter("y", [N, C], FP, isOutput=True)

    with TileContext(nc) as tc:
        with (
            tc.tile_pool(name="const", bufs=1) as constp,
            tc.tile_pool(name="xin", bufs=10) as xinp,
            tc.tile_pool(name="big", bufs=1) as bigp,
            tc.tile_pool(name="exp", bufs=3) as expp,
            tc.tile_pool(name="stg", bufs=2) as stgp,
            tc.tile_pool(name="ytmp", bufs=4) as ytmpp,
            tc.tile_pool(name="psA", bufs=1, space="PSUM") as psA,
            tc.tile_pool(name="psB", bufs=1, space="PSUM") as psB,
            tc.tile_pool(name="psV", bufs=1, space="PSUM") as psV,
        ):
            ident = constp.tile([P, P], FP, tag="ident")
            make_identity(nc, ident[:])

            # ---- persistent SBUF tensors ----
            xT = [bigp.tile([P, N], FR, tag=f"xT{cc}", name=f"xT{cc}") for cc in range(2)]
            qT = bigp.tile([64, N], FR, tag="qT")
            karr = bigp.tile([64, N], FR, tag="karr")
            vaug = [bigp.tile([P, 33 * NCH], F16, tag=f"vaug{h}", name=f"vaug{h}") for h in range(2)]
            outT = bigp.tile([64, N], F16, tag="outT")
            rden = bigp.tile([P, 8 * ITILES], FP, tag="rden")
            wq_sb = bigp.tile([P, 2, 64], FR, tag="wq")
            wk_sb = bigp.tile([P, 2, 64], FR, tag="wk")
            wv_sb = bigp.tile([P, 2, 64], FR, tag="wv")
            wo_sb = bigp.tile([64, C], F16, tag="wo")

            # ---- weight loads (gpsimd queue; DVE rounds to fp32r / fp16) ----
            wq_st = bigp.tile([P, 2, 64], FP, tag="wq_st")
            wk_st = bigp.tile([P, 2, 64], FP, tag="wk_st")
            wv_st = bigp.tile([P, 2, 64], FP, tag="wv_st")
            wo_st = bigp.tile([64, C], FP, tag="wo_st")
            for cc in range(2):
                nc.gpsimd.dma_start(out=wq_st[:, cc, :], in_=wq_d[cc * P:(cc + 1) * P, :])
                nc.gpsimd.dma_start(out=wk_st[:, cc, :], in_=wk_d[cc * P:(cc + 1) * P, :])
                nc.gpsimd.dma_start(out=wv_st[:, cc, :], in_=wv_d[cc * P:(cc + 1) * P, :])
            nc.gpsimd.dma_start(out=wo_st[:], in_=wo_d[:])
            nc.vector.tensor_copy(out=wq_sb[:], in_=wq_st[:])
            nc.vector.tensor_copy(out=wk_sb[:], in_=wk_st[:])
            nc.vector.tensor_copy(out=wv_sb[:], in_=wv_st[:])
            nc.vector.tensor_copy(out=wo_sb[:], in_=wo_st[:])

            # ---- x load + transpose to xT; qkv builds interleaved ----
            # Emission order per psum ring is chosen so each build
            # generation's ring-WAR predecessor matches its data deps.
            # Bulk psum->sbuf copies split across DVE and Pool queues.
            def x_round(pool, tag, nks):
                L = 2048 if tag == "A" else 1536
                slab = pool.tile([P, L], FP, tag=tag)
                for i, nk in enumerate(nks):
                    xt = xinp.tile([P, C], FP, tag="xt")
                    dmae = nc.sync if nk % 2 == 0 else nc.gpsimd
                    dmae.dma_start(out=xt[:], in_=x_d[P * nk:P * (nk + 1), :])
                    for cc in range(2):
                        nc.tensor.transpose(
                            slab[:, 256 * i + P * cc: 256 * i + P * (cc + 1)],
                            xt[:, P * cc:P * (cc + 1)],
                            ident[:],
                        )
                # copy halves; cc0 via the idle Act engine, cc1 via DVE
                n = len(nks)
                sv = slab[:].rearrange("p (k c f) -> p k c f", c=2, f=P)
                h1 = n // 2
                for lo, hi in ((0, h1), (h1, n)):
                    nc.scalar.copy(
                        out=xT[0][:, P * (nks[0] + lo): P * (nks[0] + hi)],
                        in_=sv[:, lo:hi, 0, :],
                    )
                    nc.vector.tensor_copy(
                        out=xT[1][:, P * (nks[0] + lo): P * (nks[0] + hi)],
                        in_=sv[:, lo:hi, 1, :],
                    )

            def proj_mms(slab, w_sb, r, it):
                for cc in range(2):
                    nc.tensor.matmul(
                        slab[0:64, 512 * r: 512 * (r + 1)],
                        lhsT=w_sb[:, cc, :],
                        rhs=xT[cc][:, 512 * it: 512 * (it + 1)],
                        start=(cc == 0), stop=(cc == 1),
                    )

            def proj_round(w_sb, dst, pool, tag, it0, nits, ceng, split_first=False):
                # dst[64, cols] = (x @ w)^T for both heads, via psum [0:64, :]
                L = 2048 if tag == "A" else 1536
                slab = pool.tile([P, L], FP, tag=tag)
                for r in range(nits):
                    proj_mms(slab, w_sb, r, it0 + r)
                def ccopy(out, in_):
                    if ceng is nc.scalar:
                        nc.scalar.copy(out=out, in_=in_)
                    else:
                        ceng.tensor_copy(out=out, in_=in_)
                if split_first and nits > 1:
                    ccopy(dst[:, 512 * it0: 512 * (it0 + 1)], slab[0:64, 0:512])
                    ccopy(
                        dst[:, 512 * (it0 + 1): 512 * (it0 + nits)],
                        slab[0:64, 512: 512 * nits],
                    )
                else:
                    ccopy(
                        dst[:, 512 * it0: 512 * (it0 + nits)],
                        slab[0:64, : 512 * nits],
                    )

            def it7_combo():
                # last i-tile of qT and karr share one B generation
                slab = psB.tile([P, 1536], FP, tag="B")
                proj_mms(slab, wq_sb, 0, 7)
                proj_mms(slab, wk_sb, 1, 7)
                nc.vector.tensor_copy(out=qT[:, 512 * 7:], in_=slab[0:64, 0:512])
                nc.scalar.copy(out=karr[:, 512 * 7:], in_=slab[0:64, 512:1024])

            def v_build(pool, tag, k0, nk):
                # both heads at once: psum [128, 64] chunks packed in a slab
                L = 2048 if tag == "A" else 1536
                slab = pool.tile([P, L], FP, tag=tag)
                for i in range(nk):
                    k = k0 + i
                    for cc in range(2):
                        nc.tensor.matmul(
                            slab[:, 64 * i: 64 * (i + 1)],
                            lhsT=xT[cc][:, P * k: P * (k + 1)],
                            rhs=wv_sb[:, cc, :],
                            start=(cc == 0), stop=(cc == 1),
                        )
                sv = slab[:].rearrange("p (k d) -> p k d", d=64)
                for h in range(2):
                    vv = vaug[h][:].rearrange("p (k e) -> p k e", e=33)
                    nc.vector.tensor_copy(
                        out=vv[:, k0:k0 + nk, 0:32],
                        in_=sv[:, 0:nk, 32 * h: 32 * (h + 1)],
                    )

            for h in range(2):
                vv = vaug[h][:].rearrange("p (k e) -> p k e", e=33)
                nc.vector.memset(vv[:, :, 32], 1.0)
            x_round(psA, "A", list(range(0, 8)))
            x_round(psB, "B", list(range(8, 14)))
            x_round(psA, "A", list(range(14, 22)))
            x_round(psB, "B", list(range(22, 28)))
            x_round(psA, "A", list(range(28, 32)))
            proj_round(wq_sb, qT[:], psA, "A", 0, 4, nc.vector, split_first=True)
            proj_round(wk_sb, karr[:], psA, "A", 0, 4, nc.scalar, split_first=True)
            proj_round(wq_sb, qT[:], psB, "B", 4, 3, nc.vector)
            proj_round(wk_sb, karr[:], psB, "B", 4, 3, nc.scalar)
            it7_combo()
            v_build(psA, "A", 0, 24)
            v_build(psB, "B", 24, 8)
            # ---- attention ----
            # per (h, it): sim slabs -> exp (fp16) -> attn@v accumulating
            # av[128i, 33]x4 blocks in the V bank (den in col 32); then
            # recip+scale (DVE), PE-transpose into outT[32h:+32, i-tile].
            # y(it-1) rides the B ring right after g7 (the B ring has two
            # A-exps of slack at each tile boundary, so this adds no Act
            # bubble); yo copies split DVE/Pool.
            def y_proj(it):
                i0 = 512 * it
                yslab = psB.tile([P, 1536], FP, tag="B")
                for m in range(4):
                    nc.tensor.matmul(
                        yslab[:, 256 * m: 256 * (m + 1)],
                        lhsT=outT[0:64, i0 + P * m: i0 + P * (m + 1)],
                        rhs=wo_sb[:],
                        start=True, stop=True, skip_group_check=True,
                    )
                for m in range(4):
                    yo = ytmpp.tile([P, C], FP, tag="yo")
                    nc.vector.tensor_copy(out=yo[:], in_=yslab[:, 256 * m: 256 * (m + 1)])
                    nc.sync.dma_start(
                        out=y_d[i0 + P * m: i0 + P * (m + 1), :], in_=yo[:]
                    )

            def attention(h, with_y):
                vv = vaug[h][:].rearrange("p (k e) -> p k e", e=33)
                tpos = None if h == 0 else (32, 0)
                for it in range(ITILES):
                    i0 = 512 * it
                    # V bank tile: cols 0-131 av (4 blocks x 33), 132-259
                    # transpose scratch; disjoint byte ranges within one gen
                    vt = psV.tile([P, 260], FP, tag="V")
                    avt = vt[:, 0:132]
                    av = avt.rearrange("p (m e) -> p m e", e=33)
                    cstart = 0
                    for gi, gsz in enumerate(GROUPS):
                        pool, tag, L = (psA, "A", 2048) if gsz == 4 else (psB, "B", 1536)
                        slab = pool.tile([P, L], FP, tag=tag)
                        for r in range(gsz):
                            c = cstart + r
                            nc.tensor.matmul(
                                slab[:, 512 * r: 512 * (r + 1)],
                                lhsT=karr[32 * h: 32 * (h + 1), P * c: P * (c + 1)],
                                rhs=qT[32 * h: 32 * (h + 1), i0: i0 + 512],
                                start=True, stop=True, tile_position=tpos,
                            )
                        eslab = expp.tile([P, L], F16, tag="E")
                        nc.scalar.activation(eslab[:], slab[:], AF.Exp)
                        for r in range(gsz):
                            c = cstart + r
                            for m in range(4):
                                nc.tensor.matmul(
                                    avt[:, 33 * m: 33 * (m + 1)],
                                    lhsT=eslab[:, 512 * r + P * m: 512 * r + P * (m + 1)],
                                    rhs=vv[:, c, :],
                                    start=(c == 0 and m == 0),
                                    stop=(c == NCH - 1 and m == 3),
                                    skip_group_check=True,
                                )
                        cstart += gsz
                        if gi == 7 and with_y and it > 0:
                            y_proj(it - 1)
                    # post: reciprocal of dens, normalize, transpose to outT
                    rd = rden[:, 8 * it + 4 * h: 8 * it + 4 * h + 4]
                    nc.vector.reciprocal(out=rd, in_=av[:, :, 32])
                    stg = stgp.tile([P, P], FP, tag="s")
                    for m in range(4):
                        nc.vector.tensor_scalar_mul(
                            stg[:, 32 * m: 32 * (m + 1)], av[:, m, 0:32],
                            rd[:, m: m + 1],
                        )
                    nc.tensor.matmul(
                        vt[:, 132:260], lhsT=stg[:], rhs=ident[:],
                        is_transpose=True, start=True, stop=True,
                        skip_group_check=True,
                    )
                    for m in range(4):
                        nc.vector.tensor_copy(
                            out=outT[32 * h: 32 * h + 32, i0 + P * m: i0 + P * (m + 1)],
                            in_=vt[32 * m: 32 * (m + 1), 132:260],
                        )

            attention(0, with_y=False)
            attention(1, with_y=True)
            y_proj(ITILES - 1)

    _split_excess_waits(nc, mybir)
    return nc


def _split_excess_waits(nc, mybir, maxw=1, carrier_cap=1):
    """walrus codegen allows few semaphore waits per engine instruction.

    Tile's scheduler can emit 3-4 on one matmul. Hoist the excess onto
    InstEventSemaphore carriers inserted immediately before the instruction
    on the same engine queue (queue is FIFO, so waiting in the carrier is
    equivalent; no reordering so no deadlock risk).
    """
    skip = {
        "InstEventSemaphore", "InstCall",
        "InstUnconditionalBranch", "InstISA", "InstRegisterMove",
    }
    for f in nc.m.functions:
        for blk in f.blocks:
            idx = 0
            while idx < len(blk.instructions):
                ins = blk.instructions[idx]
                si = getattr(ins, "sync_info", None)
                if (
                    si is not None and si.on_wait and len(si.on_wait) > maxw
                    and type(ins).__name__ not in skip
                ):
                    waits = list(si.on_wait)
                    keep, excess = waits[:maxw], waits[maxw:]
                    n_ins = 0
                    for i in range(0, len(excess), carrier_cap):
                        ev = mybir.InstEventSemaphore(
                            name=nc.get_next_instruction_name(),
                            engine=ins.engine,
                            ins=[], outs=[],
                            sync_info=mybir.SyncInfo(
                                on_wait=excess[i:i + carrier_cap], on_update=[]
                            ),
                        )
                        nc.register_instruction(ev)
                        blk.instructions.insert(idx + n_ins, ev)
                        n_ins += 1
                    ins.sync_info = mybir.SyncInfo(
                        on_wait=keep, on_update=list(si.on_update or [])
                    )
                    idx += n_ins
                idx += 1
    return nc


def get_nc():
    if "nc" not in _CACHED:
        _CACHED["nc"] = _build_nc()
    return _CACHED["nc"]


def make_in_maps(x, w_qkv, w_out):
    """Host-side sharding: core c -> batch c//2, heads (c%2)*2, (c%2)*2+1."""
    B = x.shape[0]
    xf = np.ascontiguousarray(x.reshape(B, N, C))
    scale = DH ** -0.5
    in_maps = []
    for core in range(8):
        b, hp = core // 2, core % 2
        h0, h1 = 2 * hp, 2 * hp + 1
        wq = np.concatenate(
            [w_qkv[:, h * DH:(h + 1) * DH] * scale for h in (h0, h1)], axis=1
        )  # [256, 64]
        wk = np.concatenate(
            [w_qkv[:, 128 + h * DH: 128 + (h + 1) * DH] for h in (h0, h1)], axis=1
        )  # [256, 64]
        wv = np.concatenate(
            [w_qkv[:, 256 + h * DH: 256 + (h + 1) * DH] for h in (h0, h1)], axis=1
        )  # [256, 64]
        wo = np.concatenate(
            [w_out[h * DH:(h + 1) * DH, :] for h in (h0, h1)], axis=0
        )  # [64, 256]
        in_maps.append({
            "x": np.ascontiguousarray(xf[b]),
            "wq": np.ascontiguousarray(wq.astype(np.float32)),
            "wk": np.ascontiguousarray(wk.astype(np.float32)),
            "wv": np.ascontiguousarray(wv.astype(np.float32)),
            "wo": np.ascontiguousarray(wo.astype(np.float32)),
        })
    return in_maps


def kernel(x, w_qkv, w_out, b_out):
    from concourse.bass_utils import run_bass_kernel_spmd

    nc = get_nc()
    in_maps = make_in_maps(
        np.asarray(x, dtype=np.float32),
        np.asarray(w_qkv, dtype=np.float32),
        np.asarray(w_out, dtype=np.float32),
    )
    res = run_bass_kernel_spmd(nc, in_maps, list(range(8))).results
    B, H, W = 4, 64, 64
    y = np.empty((B, N, C), dtype=np.float32)
    for b in range(B):
        y[b] = res[2 * b]["y"] + res[2 * b + 1]["y"]
    y += np.asarray(b_out, dtype=np.float32)
    return y.reshape(B, H, W, C)
